# revision 1
# baseline (speedup 1.0000x reference)
"""MoDL recon (one unroll) Trainium2 Bass kernel.

B=8 batch elements sharded 1-per-core across 8 NeuronCores (pure data
parallel).  Per core: SENSE adjoint, 3-layer CNN denoiser, and 6 CG
iterations on the SENSE normal operator, for C=12 coils on 320x320 complex
images.

All 2D centered FFTs are computed as dense DFT matmuls on the tensor engine
in fp32r using the two-pass "image stationary" trick:
    U  = MM(X, G)  = X.T @ G        (G symmetric centered DFT matrix)
    K2 = MM(U, G)  = U.T @ G = G X G = fftc(X)
which needs no explicit transposes.  ifftc uses conj(G).

Host-side work is layout only: de-interleaving re/im planes, row-tiling
[320,320] -> [128,3,320] (rows r = t*128+p, zero padded), packing conv
weights, and building the conv1 im2col stack (pure data replication).
"""

import os
import numpy as np
import ml_dtypes

N = 320
NT = 3
TS = [(0, 128), (128, 128), (256, 64)]   # (row0, rows) per row-tile
C = int(os.environ.get("K_COILS", "12"))
CG_ITERS = int(os.environ.get("K_CG", "6"))
DO_CONV = os.environ.get("K_CONV", "1") == "1"
DO_ADJ = os.environ.get("K_ADJ", "1") == "1"
L2LAM = 0.05

# conv band structure
BAND = 20
NBAND = N // BAND
W2 = N + 2  # padded width 322

_cache = {}


# ----------------------------------------------------------------------
# host-side helpers
# ----------------------------------------------------------------------

def centered_dft_matrix(n):
    F = np.fft.fft(np.eye(n), norm="ortho", axis=0)
    s = np.fft.fftshift(np.eye(n), axes=0)
    si = np.fft.ifftshift(np.eye(n), axes=0)
    return (s @ F @ si).astype(np.complex128)


def tile_rows(x):
    """[..., 320, n] -> [128, ..., 3, n] with rows r = t*128+p, zero pad."""
    lead = x.shape[:-2]
    n = x.shape[-1]
    xp = np.zeros(lead + (384, n), dtype=x.dtype)
    xp[..., :320, :] = x
    xp = xp.reshape(lead + (3, 128, n))          # [..., t, p, n]
    # move p to front
    xp = np.moveaxis(xp, -2, 0)                  # [128, ..., t, n]
    return np.ascontiguousarray(xp)


def untile_rows(x):
    """[128, 3, n] -> [320, n]"""
    # x[p, t, n] -> rows t*128+p
    out = np.transpose(x, (1, 0, 2)).reshape(384, x.shape[-1])
    return out[:320]


def host_prep(inputs):
    x = inputs["x"]
    maps = inputs["maps"]
    masks = inputs["masks"]
    ksp = inputs["ksp"]
    w1, b1 = inputs["w1"], inputs["b1"]
    w2, b2 = inputs["w2"], inputs["b2"]
    w3, b3 = inputs["w3"], inputs["b3"]
    B = x.shape[0]

    G = centered_dft_matrix(N)
    gpl = np.stack([G.real, G.imag, -G.imag]).astype(np.float32)  # [3,320,320]
    g3d = tile_rows(gpl)   # [128, 3(plane), 3(t), 320]

    # conv weights
    w1s = np.zeros((18, 64), np.float32)
    for dy in range(3):
        for dx in range(3):
            off = dy * 3 + dx
            for ci in range(2):
                w1s[off * 2 + ci, :] = w1[:, ci, dy, dx]
    w2p = np.zeros((128, 3, 64), np.float32)
    w2q = np.zeros((128, 3, 64), np.float32)
    w3p = np.zeros((128, 3, 2), np.float32)
    w3q = np.zeros((128, 3, 2), np.float32)
    for dy in range(3):
        w2p[0:64, dy, :] = w2[:, :, dy, 1].T     # center tap reads base half
        w2p[64:128, dy, :] = w2[:, :, dy, 0].T   # left tap reads dup (-1) half
        w2q[0:64, dy, :] = w2[:, :, dy, 2].T     # right tap: base half at +1
        w3p[0:64, dy, :] = w3[:, :, dy, 1].T
        w3p[64:128, dy, :] = w3[:, :, dy, 0].T
        w3q[0:64, dy, :] = w3[:, :, dy, 2].T

    bf = ml_dtypes.bfloat16
    shared = {
        "g3d": g3d,
        "w1s": w1s.astype(bf),
        "w2p": np.ascontiguousarray(w2p.astype(bf)),
        "w2q": np.ascontiguousarray(w2q.astype(bf)),
        "w3p": np.ascontiguousarray(w3p.astype(bf)),
        "w3q": np.ascontiguousarray(w3q.astype(bf)),
        "b1v": b1.reshape(64, 1).astype(np.float32),
        "b2v": b2.reshape(64, 1).astype(np.float32),
        "b3v": b3.reshape(2, 1).astype(np.float32),
    }

    per_core = []
    for b in range(B):
        xpl = np.transpose(x[b], (2, 0, 1)).astype(np.float32)      # [2,320,320]
        mpl = np.transpose(maps[b], (3, 0, 1, 2)).astype(np.float32)  # [2,12,320,320]
        mpl = np.transpose(mpl, (1, 0, 2, 3))                        # [12,2,320,320]
        kpl = np.transpose(ksp[b], (3, 0, 1, 2)).astype(np.float32)
        kpl = np.transpose(kpl, (1, 0, 2, 3))

        # conv1 stack: [18, 320*322] bf16
        xq = np.zeros((2, W2, W2 + 2), np.float32)
        xq[:, 1:321, 2:322] = xpl
        stack = np.zeros((18, N * W2), np.float32)
        for dy in range(3):
            for dx in range(3):
                off = dy * 3 + dx
                for ci in range(2):
                    stack[off * 2 + ci] = xq[ci, dy:dy + N, dx:dx + W2].reshape(-1)

        per_core.append({
            "xt": tile_rows(xpl),                         # [128,2,3,320]
            "mapst": tile_rows(mpl),                      # [128,12,2,3,320]
            "kspt": tile_rows(kpl),
            "maskt": tile_rows(masks[b].astype(np.float32)),  # [128,3,320]
            "stackh": stack.astype(bf),
            **shared,
        })
    return per_core


# ----------------------------------------------------------------------
# device program
# ----------------------------------------------------------------------

def build_program():
    import concourse.bass as bass
    import concourse.mybir as mybir
    import concourse.tile as tile
    from concourse import bacc, bass_isa
    from contextlib import ExitStack

    f32 = mybir.dt.float32
    f32r = mybir.dt.float32r
    bf16 = mybir.dt.bfloat16
    AL = mybir.AluOpType
    AF = mybir.ActivationFunctionType

    nc = bacc.Bacc("TRN2", target_bir_lowering=False)

    # DRAM tensors
    xt_d = nc.dram_tensor("xt", [128, 2, 3, N], f32, kind="ExternalInput")
    mapst_d = nc.dram_tensor("mapst", [128, 12, 2, 3, N], f32, kind="ExternalInput")
    kspt_d = nc.dram_tensor("kspt", [128, 12, 2, 3, N], f32, kind="ExternalInput")
    maskt_d = nc.dram_tensor("maskt", [128, 3, N], f32, kind="ExternalInput")
    g3d_d = nc.dram_tensor("g3d", [128, 3, 3, N], f32, kind="ExternalInput")
    stackh_d = nc.dram_tensor("stackh", [18, N * W2], bf16, kind="ExternalInput")
    w1s_d = nc.dram_tensor("w1s", [18, 64], bf16, kind="ExternalInput")
    w2p_d = nc.dram_tensor("w2p", [128, 3, 64], bf16, kind="ExternalInput")
    w2q_d = nc.dram_tensor("w2q", [128, 3, 64], bf16, kind="ExternalInput")
    w3p_d = nc.dram_tensor("w3p", [128, 3, 2], bf16, kind="ExternalInput")
    w3q_d = nc.dram_tensor("w3q", [128, 3, 2], bf16, kind="ExternalInput")
    b1v_d = nc.dram_tensor("b1v", [64, 1], f32, kind="ExternalInput")
    b2v_d = nc.dram_tensor("b2v", [64, 1], f32, kind="ExternalInput")
    b3v_d = nc.dram_tensor("b3v", [2, 1], f32, kind="ExternalInput")
    xot_d = nc.dram_tensor("xot", [128, 2, 3, N], f32, kind="ExternalOutput")

    # complex-matmul recipes: list per out-plane of (stat_plane, g_plane) terms
    # g planes: 0=Gr, 1=Gi, 2=-Gi
    FWD = [[(0, 0), (1, 2)], [(0, 1), (1, 0)]]
    INV = [[(0, 0), (1, 1)], [(0, 2), (1, 0)]]

    with tile.TileContext(nc) as tc, ExitStack() as topstack:
        const = topstack.enter_context(tc.tile_pool(name="const", bufs=1))
        ps = topstack.enter_context(tc.tile_pool(name="ps", bufs=8, space="PSUM"))
        sc = topstack.enter_context(tc.tile_pool(name="sc", bufs=32))

        # --- constants + state ------------------------------------------------
        gr3 = const.tile([128, 3, 3, N], f32r)
        mask_t = const.tile([128, 3, N], f32)
        x_t = const.tile([128, 2, 3, N], f32)
        rhs_t = const.tile([128, 2, 3, N], f32)
        r_t = const.tile([128, 2, 3, N], f32)
        p_t = const.tile([128, 2, 3, N], f32)
        acc_t = const.tile([128, 2, 3, N], f32)
        w1s_t = const.tile([18, 64], bf16)
        w2p_t = const.tile([128, 3, 64], bf16)
        w2q_t = const.tile([128, 3, 64], bf16)
        w3p_t = const.tile([128, 3, 2], bf16)
        w3q_t = const.tile([128, 3, 2], bf16)
        b1v_t = const.tile([64, 1], f32)
        b2v_t = const.tile([64, 1], f32)
        b3v_t = const.tile([2, 1], f32)

        nc.sync.dma_start(mask_t[:], maskt_d[:, :, :])
        nc.sync.dma_start(x_t[:], xt_d[:, :, :, :])
        nc.sync.dma_start(w1s_t[:], w1s_d[:, :])
        nc.sync.dma_start(w2p_t[:], w2p_d[:, :, :])
        nc.sync.dma_start(w2q_t[:], w2q_d[:, :, :])
        nc.sync.dma_start(w3p_t[:], w3p_d[:, :, :])
        nc.sync.dma_start(w3q_t[:], w3q_d[:, :, :])
        nc.sync.dma_start(b1v_t[:], b1v_d[:, :])
        nc.sync.dma_start(b2v_t[:], b2v_d[:, :])
        nc.sync.dma_start(b3v_t[:], b3v_d[:, :])

        # round G into fp32r via a scoped staging tile
        with tc.tile_pool(name="ground", bufs=1) as gpool:
            gsrc = gpool.tile([128, 3, 3, N], f32)
            nc.sync.dma_start(gsrc[:], g3d_d[:, :, :, :])
            nc.vector.tensor_copy(gr3[:, :, :, :], gsrc[:, :, :, :])

        # DRAM staging for conv output (residual term), bf16
        dram = topstack.enter_context(tc.tile_pool(name="dram", bufs=1, space="DRAM"))
        o3stage = dram.tile([2, N, N], bf16)

        # --- denoiser conv (bf16, banded) ------------------------------------
        if DO_CONV:
            with tc.tile_pool(name="cstk", bufs=2) as cstk, \
                 tc.tile_pool(name="ch1", bufs=2) as ch1, \
                 tc.tile_pool(name="ch2", bufs=2) as ch2, \
                 tc.tile_pool(name="co3", bufs=2) as co3:
                for bd in range(NBAND):
                    s = bd * BAND
                    h1s, h1e = s - 2, s + BAND + 2        # h1 rows window [h1s,h1e) len 24
                    h2s, h2e = s - 1, s + BAND + 1        # h2 rows window len 22
                    v0 = max(0, -h1s)
                    v1 = 24 - max(0, h1e - N)
                    w0 = max(0, -h2s)
                    w1_ = 22 - max(0, h2e - N)

                    L1 = 24 * W2
                    L2L = 22 * W2
                    stk = cstk.tile([18, L1 + 8], bf16, tag="stk")
                    h1q = ch1.tile([128, L1 + 8], bf16, tag="h1q")
                    h2q = ch2.tile([128, L2L + 8], bf16, tag="h2q")
                    o3b = co3.tile([2, BAND * W2], bf16, tag="o3b")

                    span = (v1 - v0) * W2
                    nc.sync.dma_start(
                        stk[:18, 0:span],
                        stackh_d[:, (h1s + v0) * W2:(h1s + v0) * W2 + span],
                    )
                    # conv1
                    for j in range(0, span, 512):
                        n = min(512, span - j)
                        pt = ps.tile([128, 512], f32, tag="ps")
                        nc.tensor.matmul(pt[:64, :n], w1s_t[:, :], stk[:18, j:j + n],
                                         start=True, stop=True)
                        nc.scalar.activation(h1q[0:64, v0 * W2 + j:v0 * W2 + j + n],
                                             pt[:64, :n], AF.Relu, bias=b1v_t[:, :])
                    # zero invalid rows / pad cols / slack
                    if v0 > 0:
                        nc.gpsimd.memset(h1q[0:64, 0:v0 * W2], 0.0)
                    if v1 < 24:
                        nc.gpsimd.memset(h1q[0:64, v1 * W2:L1], 0.0)
                    nc.gpsimd.memset(h1q[0:64, L1:L1 + 8], 0.0)
                    h1v = h1q[0:64, 0:L1].rearrange("p (r x) -> p r x", x=W2)
                    nc.gpsimd.memset(h1v[:, :, 0:1], 0.0)
                    nc.gpsimd.memset(h1v[:, :, W2 - 1:W2], 0.0)
                    # dup shifted -1 into partitions 64:128 (dup[m]=base[m-1])
                    nc.sync.dma_start(h1q[64:128, 1:L1 + 8], h1q[0:64, 0:L1 + 7])
                    nc.gpsimd.memset(h1q[64:128, 0:1], 0.0)

                    # conv2: chunks over valid h2 rows [w0,w1)
                    for j in range(w0 * W2, w1_ * W2, 512):
                        n = min(512, w1_ * W2 - j)
                        pt = ps.tile([128, 512], f32, tag="ps")
                        k = 0
                        for dy in range(3):
                            nc.tensor.matmul(pt[:64, :n], w2p_t[:, dy, :],
                                             h1q[:, j + dy * W2:j + dy * W2 + n],
                                             start=(k == 0), stop=False)
                            k += 1
                        for dy in range(3):
                            k += 1
                            nc.tensor.matmul(pt[:64, :n], w2q_t[:, dy, :],
                                             h1q[:, j + dy * W2 + 1:j + dy * W2 + 1 + n],
                                             start=False, stop=(k == 6))
                        nc.scalar.activation(h2q[0:64, j:j + n], pt[:64, :n],
                                             AF.Relu, bias=b2v_t[:, :])
                    if w0 > 0:
                        nc.gpsimd.memset(h2q[0:64, 0:w0 * W2], 0.0)
                    if w1_ < 22:
                        nc.gpsimd.memset(h2q[0:64, w1_ * W2:L2L], 0.0)
                    nc.gpsimd.memset(h2q[0:64, L2L:L2L + 8], 0.0)
                    h2v = h2q[0:64, 0:L2L].rearrange("p (r x) -> p r x", x=W2)
                    nc.gpsimd.memset(h2v[:, :, 0:1], 0.0)
                    nc.gpsimd.memset(h2v[:, :, W2 - 1:W2], 0.0)
                    nc.sync.dma_start(h2q[64:128, 1:L2L + 8], h2q[0:64, 0:L2L + 7])
                    nc.gpsimd.memset(h2q[64:128, 0:1], 0.0)

                    # conv3: output rows [s, s+BAND)
                    for j in range(0, BAND * W2, 512):
                        n = min(512, BAND * W2 - j)
                        pt = ps.tile([128, 512], f32, tag="ps")
                        k = 0
                        for dy in range(3):
                            nc.tensor.matmul(pt[:2, :n], w3p_t[:, dy, :],
                                             h2q[:, j + dy * W2:j + dy * W2 + n],
                                             start=(k == 0), stop=False)
                            k += 1
                        for dy in range(3):
                            k += 1
                            nc.tensor.matmul(pt[:2, :n], w3q_t[:, dy, :],
                                             h2q[:, j + dy * W2 + 1:j + dy * W2 + 1 + n],
                                             start=False, stop=(k == 6))
                        nc.scalar.activation(o3b[0:2, j:j + n], pt[:2, :n],
                                             AF.Identity, bias=b3v_t[:, :])
                    o3v = o3b[0:2, :].rearrange("c (r x) -> c r x", x=W2)
                    nc.sync.dma_start(o3stage[:, s:s + BAND, :], o3v[:, :, 1:N + 1])

        # --- rhs partial: rhs = lam*(x + o3) ---------------------------------
        with tc.tile_pool(name="o3g", bufs=1) as o3g:
            o3t = o3g.tile([128, 2, 3, N], bf16)
            o3f = o3g.tile([128, 2, 3, N], f32)
            nc.gpsimd.memset(o3t[:, :, :, :], 0.0)
            if DO_CONV:
                for ch in range(2):
                    for t in range(2):
                        nc.sync.dma_start(
                            o3t[:, ch, t, :],
                            o3stage[ch, t * 128:(t + 1) * 128, :])
                    nc.sync.dma_start(
                        o3t[:64, ch, 2, :], o3stage[ch, 256:320, :])
            nc.vector.tensor_copy(o3f[:, :, :, :], o3t[:, :, :, :])
            for pl in range(2):
                nc.scalar.mul(rhs_t[:, pl], x_t[:, pl], L2LAM)
                nc.vector.scalar_tensor_tensor(
                    rhs_t[:, pl], o3f[:, pl], L2LAM, rhs_t[:, pl],
                    op0=AL.mult, op1=AL.add)

        # --- maps ------------------------------------------------------------
        mpool = topstack.enter_context(tc.tile_pool(name="maps", bufs=1))
        maps_t = mpool.tile([128, 12, 2, 3, N], f32)
        for c in range(12):
            nc.sync.dma_start(maps_t[:, c], mapst_d[:, c])

        # --- working pools ---------------------------------------------------
        work = topstack.enter_context(tc.tile_pool(name="work", bufs=4))
        tg_p = topstack.enter_context(tc.tile_pool(name="tg", bufs=2))
        td_p = topstack.enter_context(tc.tile_pool(name="td", bufs=6))
        scr_p = topstack.enter_context(tc.tile_pool(name="scr", bufs=1))

        def pass_mm(stat, recipe, evac):
            """one complex 1D DFT pass: out = stat.T @ Gc; evac(pl, mt, M, psum)"""
            for mt, (m0, M) in enumerate(TS):
                for pl in range(2):
                    pt = ps.tile([128, 512], f32, tag="ps")
                    k = 0
                    for (sp, gp) in recipe[pl]:
                        for kt, (k0, K) in enumerate(TS):
                            nc.tensor.matmul(
                                pt[:M, :N],
                                stat[:K, sp, kt, m0:m0 + M],
                                gr3[:K, gp, kt, :],
                                start=(k == 0), stop=(k == 5))
                            k += 1
                    evac(pl, mt, M, pt)

        def coil_mult(src, c):
            A = work.tile([128, 2, 3, N], f32r, tag="work")
            mr = maps_t[:, c, 0]
            mi = maps_t[:, c, 1]
            ta = tg_p.tile([128, 3, N], f32, tag="tg")
            tb = tg_p.tile([128, 3, N], f32, tag="tg")
            nc.gpsimd.tensor_tensor(ta[:], mr, src[:, 0], AL.mult)
            nc.gpsimd.tensor_tensor(tb[:], mi, src[:, 1], AL.mult)
            nc.gpsimd.tensor_tensor(A[:, 0], ta[:], tb[:], AL.subtract)
            tc_ = tg_p.tile([128, 3, N], f32, tag="tg")
            td = tg_p.tile([128, 3, N], f32, tag="tg")
            nc.gpsimd.tensor_tensor(tc_[:], mr, src[:, 1], AL.mult)
            nc.gpsimd.tensor_tensor(td[:], mi, src[:, 0], AL.mult)
            nc.gpsimd.tensor_tensor(A[:, 1], tc_[:], td[:], AL.add)
            return A

        def final_pass(stat, recipe, c, acc):
            """last DFT pass: emit both plane groups per mt, then combine
            conj(maps[c]) * V into acc directly from PSUM."""
            for mt, (m0, M) in enumerate(TS):
                pts = []
                for pl in range(2):
                    pt = ps.tile([128, 512], f32, tag="ps")
                    k = 0
                    for (sp, gp) in recipe[pl]:
                        for kt, (k0, K) in enumerate(TS):
                            nc.tensor.matmul(
                                pt[:M, :N],
                                stat[:K, sp, kt, m0:m0 + M],
                                gr3[:K, gp, kt, :],
                                start=(k == 0), stop=(k == 5))
                            k += 1
                    pts.append(pt)
                mr = maps_t[:M, c, 0, mt, :]
                mi = maps_t[:M, c, 1, mt, :]
                vr, vi = pts[0][:M, :N], pts[1][:M, :N]
                t1 = td_p.tile([128, N], f32, tag="td")
                t2 = td_p.tile([128, N], f32, tag="td")
                nc.vector.tensor_tensor(t1[:M], vr, mr, AL.mult)
                nc.vector.tensor_tensor(t2[:M], vi, mi, AL.mult)
                nc.vector.tensor_tensor(t1[:M], t1[:M], t2[:M], AL.add)
                nc.vector.tensor_tensor(acc[:M, 0, mt, :], acc[:M, 0, mt, :],
                                        t1[:M], AL.add)
                t3 = td_p.tile([128, N], f32, tag="td")
                t4 = td_p.tile([128, N], f32, tag="td")
                nc.vector.tensor_tensor(t3[:M], vi, mr, AL.mult)
                nc.vector.tensor_tensor(t4[:M], vr, mi, AL.mult)
                nc.vector.tensor_tensor(t3[:M], t3[:M], t4[:M], AL.subtract)
                nc.vector.tensor_tensor(acc[:M, 1, mt, :], acc[:M, 1, mt, :],
                                        t3[:M], AL.add)

        def plain_evac(dst):
            def f(pl, mt, M, pt):
                if pl == 0:
                    nc.scalar.copy(dst[:M, pl, mt, :], pt[:M, :N])
                else:
                    nc.vector.tensor_copy(dst[:M, pl, mt, :], pt[:M, :N])
            return f

        def mask_evac(dst):
            def f(pl, mt, M, pt):
                nc.vector.tensor_tensor(dst[:M, pl, mt, :], pt[:M, :N],
                                        mask_t[:M, mt, :], AL.mult)
            return f


        def emit_aop(src, acc):
            for pl in range(2):
                nc.scalar.mul(acc[:, pl], src[:, pl], L2LAM)
            for c in range(C):
                A = coil_mult(src, c)
                U1 = work.tile([128, 2, 3, N], f32r, tag="work")
                pass_mm(A, FWD, plain_evac(U1))
                K2m = work.tile([128, 2, 3, N], f32r, tag="work")
                pass_mm(U1, FWD, mask_evac(K2m))
                U2 = work.tile([128, 2, 3, N], f32r, tag="work")
                pass_mm(K2m, INV, plain_evac(U2))
                final_pass(U2, INV, c, acc)

        # --- adjoint into rhs -------------------------------------------------
        if DO_ADJ:
            with tc.tile_pool(name="kspp", bufs=1) as kspp:
                for c in range(C):
                    kt = kspp.tile([128, 2, 3, N], f32, tag="ksp")
                    nc.sync.dma_start(kt[:], kspt_d[:, c])
                    T = work.tile([128, 2, 3, N], f32r, tag="work")
                    for pl in range(2):
                        nc.vector.tensor_tensor(T[:, pl], kt[:, pl],
                                                mask_t[:, :, :], AL.mult)
                    U2 = work.tile([128, 2, 3, N], f32r, tag="work")
                    pass_mm(T, INV, plain_evac(U2))
                    final_pass(U2, INV, c, rhs_t)

        # --- CG ----------------------------------------------------------------
        AX = mybir.AxisListType
        onesf = const.tile([128, 128], f32)
        nc.gpsimd.memset(onesf[:], 1.0)
        ones_r = const.tile([128, 128], f32r)
        nc.vector.tensor_copy(ones_r[:], onesf[:])
        d8_p = topstack.enter_context(tc.tile_pool(name="d8", bufs=6))

        def emit_dot(a, b, out):
            """out[128,1] fp32 = sum(a*b) over both planes, broadcast to all
            partitions.  Partials -> [128,8] fp32r -> ones-matmul -> reduce."""
            p8a = d8_p.tile([128, 8], f32r, tag="d8")
            p8b = d8_p.tile([128, 8], f32r, tag="d8")
            for pl, p8 in ((0, p8a), (1, p8b)):
                scrap = scr_p.tile([128, 3, N], f32, tag="scrap")
                nc.vector.tensor_tensor(scrap[:], a[:, pl], b[:, pl], AL.mult)
                v8 = scrap[:].rearrange("p t n -> p (t n)").rearrange(
                    "p (a b) -> p a b", a=8)
                with nc.allow_low_precision(reason="fp32r dot partials"):
                    nc.vector.tensor_reduce(p8[:], v8, axis=AX.X, op=AL.add)
            with nc.allow_low_precision(reason="fp32r dot partials"):
                nc.vector.tensor_tensor(p8a[:], p8a[:], p8b[:], AL.add)
            s2 = ps.tile([128, 512], f32, tag="ps")
            nc.tensor.matmul(s2[:, 0:8], ones_r[:, :], p8a[:, :],
                             start=True, stop=True)
            nc.vector.tensor_reduce(out[:], s2[:, 0:8], axis=AX.X, op=AL.add)

        # r = rhs - Aop(x); p = r; rs = <r,r>
        emit_aop(x_t, acc_t)
        for pl in range(2):
            nc.vector.tensor_tensor(r_t[:, pl], rhs_t[:, pl], acc_t[:, pl],
                                    AL.subtract)
            nc.vector.tensor_copy(p_t[:, pl], r_t[:, pl])
        rs = sc.tile([128, 1], f32, tag="sc")
        emit_dot(r_t, r_t, rs)

        for it in range(CG_ITERS):
            emit_aop(p_t, acc_t)
            pap = sc.tile([128, 1], f32, tag="sc")
            emit_dot(p_t, acc_t, pap)
            rec = sc.tile([128, 1], f32, tag="sc")
            nc.vector.reciprocal(rec[:], pap[:])
            al = sc.tile([128, 1], f32, tag="sc")
            nc.vector.tensor_tensor(al[:], rs[:], rec[:], AL.mult)
            aln = sc.tile([128, 1], f32, tag="sc")
            nc.vector.tensor_scalar_mul(aln[:], al[:], -1.0)
            for pl in range(2):
                nc.vector.scalar_tensor_tensor(
                    x_t[:, pl], p_t[:, pl], al[:], x_t[:, pl],
                    op0=AL.mult, op1=AL.add)
                nc.vector.scalar_tensor_tensor(
                    r_t[:, pl], acc_t[:, pl], aln[:], r_t[:, pl],
                    op0=AL.mult, op1=AL.add)
            rsn = sc.tile([128, 1], f32, tag="sc")
            emit_dot(r_t, r_t, rsn)
            if it < CG_ITERS - 1:
                rrec = sc.tile([128, 1], f32, tag="sc")
                nc.vector.reciprocal(rrec[:], rs[:])
                be = sc.tile([128, 1], f32, tag="sc")
                nc.vector.tensor_tensor(be[:], rsn[:], rrec[:], AL.mult)
                for pl in range(2):
                    nc.vector.scalar_tensor_tensor(
                        p_t[:, pl], p_t[:, pl], be[:], r_t[:, pl],
                        op0=AL.mult, op1=AL.add)
            rs = rsn

        nc.sync.dma_start(xot_d[:, :, :, :], x_t[:])

    nc.compile()
    return nc


# ----------------------------------------------------------------------
# entry point
# ----------------------------------------------------------------------

def kernel(**inputs):
    from concourse.bass_utils import run_bass_kernel_spmd

    B = inputs["x"].shape[0]
    per_core = host_prep(inputs)

    if "nc" not in _cache:
        _cache["nc"] = build_program()
    nc = _cache["nc"]

    res = run_bass_kernel_spmd(nc, per_core, core_ids=list(range(B)))
    out = np.zeros((B, N, N, 2), np.float32)
    for b in range(B):
        xo = res.results[b]["xot"]          # [128,2,3,320]
        out[b, :, :, 0] = untile_rows(xo[:, 0])
        out[b, :, :, 1] = untile_rows(xo[:, 1])
    return out



# revision 3
# speedup vs baseline: 1.2809x; 1.2809x over previous
"""MoDL recon (one unroll) Trainium2 Bass kernel.

B=8 batch elements sharded 1-per-core across 8 NeuronCores (pure data
parallel).  Per core: SENSE adjoint, 3-layer CNN denoiser, and 6 CG
iterations on the SENSE normal operator, for C=12 coils on 320x320 complex
images.

All 2D centered FFTs are computed as dense DFT matmuls on the tensor engine
using the two-pass "image stationary" trick:
    U  = MM(X, G)  = X.T @ G        (G symmetric centered DFT matrix)
    K2 = MM(U, G)  = U.T @ G = G X G = fftc(X)
which needs no explicit transposes.  ifftc uses conj(G).

DFT matmuls run in fp16 (1 cycle/row on the PE vs 2+ for fp32r) with the
complex contraction packed into 640 rows = 5 chunks of K=128:
chunks [re rows 0:128 | re 128:256 | im 0:128 | im 128:256 | re-tail::im-tail]
against four precomputed stacked G matrices (fwd/inv x out-real/out-imag).
5 accumulating matmuls per PSUM tile instead of 6, no K=64 stragglers.

Host-side work is layout only: de-interleaving re/im planes, row-tiling
[320,320] -> [128,3,320], packing the stacked DFT matrices / mask, packing
conv weights, and building the conv1 im2col stack.
"""

import os
import numpy as np
import ml_dtypes

N = 320
NT = 3
TS = [(0, 128), (128, 128), (256, 64)]   # (row0, rows) per row-tile
C = int(os.environ.get("K_COILS", "12"))
CG_ITERS = int(os.environ.get("K_CG", "6"))
DO_CONV = os.environ.get("K_CONV", "1") == "1"
DO_ADJ = os.environ.get("K_ADJ", "1") == "1"
L2LAM = 0.05

# conv band structure
BAND = 20
NBAND = N // BAND
W2 = N + 2  # padded width 322

# (plane, mt) -> (chunk, partition offset) in the 5-chunk stationary layout
CHUNK = {(0, 0): (0, 0), (0, 1): (1, 0), (0, 2): (4, 0),
         (1, 0): (2, 0), (1, 1): (3, 0), (1, 2): (4, 64)}

_cache = {}


# ----------------------------------------------------------------------
# host-side helpers
# ----------------------------------------------------------------------

def centered_dft_matrix(n):
    F = np.fft.fft(np.eye(n), norm="ortho", axis=0)
    s = np.fft.fftshift(np.eye(n), axes=0)
    si = np.fft.ifftshift(np.eye(n), axes=0)
    return (s @ F @ si).astype(np.complex128)


def tile_rows(x):
    """[..., 320, n] -> [128, ..., 3, n] with rows r = t*128+p, zero pad."""
    lead = x.shape[:-2]
    n = x.shape[-1]
    xp = np.zeros(lead + (384, n), dtype=x.dtype)
    xp[..., :320, :] = x
    xp = xp.reshape(lead + (3, 128, n))          # [..., t, p, n]
    # move p to front
    xp = np.moveaxis(xp, -2, 0)                  # [128, ..., t, n]
    return np.ascontiguousarray(xp)


def untile_rows(x):
    """[128, 3, n] -> [320, n]"""
    # x[p, t, n] -> rows t*128+p
    out = np.transpose(x, (1, 0, 2)).reshape(384, x.shape[-1])
    return out[:320]


def stack5(p0, p1, dtype):
    """[320,320] plane-0 + plane-1 matrices -> [128, 5, 320] chunk layout."""
    out = np.zeros((128, 5, N), np.float64)
    out[:, 0, :] = p0[0:128]
    out[:, 1, :] = p0[128:256]
    out[:, 2, :] = p1[0:128]
    out[:, 3, :] = p1[128:256]
    out[0:64, 4, :] = p0[256:320]
    out[64:128, 4, :] = p1[256:320]
    return np.ascontiguousarray(out.astype(dtype))


def host_prep(inputs):
    x = inputs["x"]
    maps = inputs["maps"]
    masks = inputs["masks"]
    ksp = inputs["ksp"]
    w1, b1 = inputs["w1"], inputs["b1"]
    w2, b2 = inputs["w2"], inputs["b2"]
    w3, b3 = inputs["w3"], inputs["b3"]
    B = x.shape[0]

    G = centered_dft_matrix(N)
    f16 = np.float16
    # stacked DFT matrices: moving operands for the packed-640 contraction
    g5 = np.stack([
        stack5(G.real, -G.imag, f16),   # fwd, real out
        stack5(G.imag, G.real, f16),    # fwd, imag out
        stack5(G.real, G.imag, f16),    # inv, real out
        stack5(-G.imag, G.real, f16),   # inv, imag out
    ])                                   # [4, 128, 5, 320]

    # conv weights
    w1s = np.zeros((18, 64), np.float32)
    for dy in range(3):
        for dx in range(3):
            off = dy * 3 + dx
            for ci in range(2):
                w1s[off * 2 + ci, :] = w1[:, ci, dy, dx]
    w2p = np.zeros((128, 3, 64), np.float32)
    w2q = np.zeros((128, 3, 64), np.float32)
    w3p = np.zeros((128, 3, 2), np.float32)
    w3q = np.zeros((128, 3, 2), np.float32)
    for dy in range(3):
        w2p[0:64, dy, :] = w2[:, :, dy, 1].T     # center tap reads base half
        w2p[64:128, dy, :] = w2[:, :, dy, 0].T   # left tap reads dup (-1) half
        w2q[0:64, dy, :] = w2[:, :, dy, 2].T     # right tap: base half at +1
        w3p[0:64, dy, :] = w3[:, :, dy, 1].T
        w3p[64:128, dy, :] = w3[:, :, dy, 0].T
        w3q[0:64, dy, :] = w3[:, :, dy, 2].T

    bf = ml_dtypes.bfloat16
    shared = {
        "g5": g5,
        "w1s": w1s.astype(bf),
        "w2p": np.ascontiguousarray(w2p.astype(bf)),
        "w2q": np.ascontiguousarray(w2q.astype(bf)),
        "w3p": np.ascontiguousarray(w3p.astype(bf)),
        "w3q": np.ascontiguousarray(w3q.astype(bf)),
        "b1v": b1.reshape(64, 1).astype(np.float32),
        "b2v": b2.reshape(64, 1).astype(np.float32),
        "b3v": b3.reshape(2, 1).astype(np.float32),
    }

    per_core = []
    for b in range(B):
        xpl = np.transpose(x[b], (2, 0, 1)).astype(np.float32)      # [2,320,320]
        mpl = np.transpose(maps[b], (3, 0, 1, 2)).astype(np.float32)  # [2,12,320,320]
        mpl = np.transpose(mpl, (1, 0, 2, 3))                        # [12,2,320,320]
        kpl = np.transpose(ksp[b], (3, 0, 1, 2)).astype(np.float32)
        kpl = np.transpose(kpl, (1, 0, 2, 3))

        # mask in 5-chunk layout (chunks 2,3,4-hi duplicate for the imag plane)
        m = masks[b].astype(np.float32)
        mask5 = np.zeros((128, 5, N), np.float32)
        mask5[:, 0, :] = m[0:128]
        mask5[:, 1, :] = m[128:256]
        mask5[:, 2, :] = m[0:128]
        mask5[:, 3, :] = m[128:256]
        mask5[0:64, 4, :] = m[256:320]
        mask5[64:128, 4, :] = m[256:320]

        # conv1 stack: [18, 320*322] bf16
        xq = np.zeros((2, W2, W2 + 2), np.float32)
        xq[:, 1:321, 2:322] = xpl
        stack = np.zeros((18, N * W2), np.float32)
        for dy in range(3):
            for dx in range(3):
                off = dy * 3 + dx
                for ci in range(2):
                    stack[off * 2 + ci] = xq[ci, dy:dy + N, dx:dx + W2].reshape(-1)

        per_core.append({
            "xt": tile_rows(xpl),                         # [128,2,3,320]
            "mapst": tile_rows(mpl),                      # [128,12,2,3,320]
            "kspt": tile_rows(kpl),
            "mask5": mask5,
            "stackh": stack.astype(bf),
            **shared,
        })
    return per_core


# ----------------------------------------------------------------------
# device program
# ----------------------------------------------------------------------

def build_program():
    import concourse.bass as bass
    import concourse.mybir as mybir
    import concourse.tile as tile
    from concourse import bacc, bass_isa
    from contextlib import ExitStack

    f32 = mybir.dt.float32
    f32r = mybir.dt.float32r
    f16 = mybir.dt.float16
    bf16 = mybir.dt.bfloat16
    AL = mybir.AluOpType
    AF = mybir.ActivationFunctionType

    nc = bacc.Bacc("TRN2", target_bir_lowering=False)

    # DRAM tensors
    xt_d = nc.dram_tensor("xt", [128, 2, 3, N], f32, kind="ExternalInput")
    mapst_d = nc.dram_tensor("mapst", [128, 12, 2, 3, N], f32, kind="ExternalInput")
    kspt_d = nc.dram_tensor("kspt", [128, 12, 2, 3, N], f32, kind="ExternalInput")
    mask5_d = nc.dram_tensor("mask5", [128, 5, N], f32, kind="ExternalInput")
    g5_d = nc.dram_tensor("g5", [4, 128, 5, N], f16, kind="ExternalInput")
    stackh_d = nc.dram_tensor("stackh", [18, N * W2], bf16, kind="ExternalInput")
    w1s_d = nc.dram_tensor("w1s", [18, 64], bf16, kind="ExternalInput")
    w2p_d = nc.dram_tensor("w2p", [128, 3, 64], bf16, kind="ExternalInput")
    w2q_d = nc.dram_tensor("w2q", [128, 3, 64], bf16, kind="ExternalInput")
    w3p_d = nc.dram_tensor("w3p", [128, 3, 2], bf16, kind="ExternalInput")
    w3q_d = nc.dram_tensor("w3q", [128, 3, 2], bf16, kind="ExternalInput")
    b1v_d = nc.dram_tensor("b1v", [64, 1], f32, kind="ExternalInput")
    b2v_d = nc.dram_tensor("b2v", [64, 1], f32, kind="ExternalInput")
    b3v_d = nc.dram_tensor("b3v", [2, 1], f32, kind="ExternalInput")
    xot_d = nc.dram_tensor("xot", [128, 2, 3, N], f32, kind="ExternalOutput")

    FWD = (0, 1)   # g5 indices for the two out planes
    INV = (2, 3)

    with tile.TileContext(nc) as tc, ExitStack() as topstack:
        const = topstack.enter_context(tc.tile_pool(name="const", bufs=1))
        ps = topstack.enter_context(tc.tile_pool(name="ps", bufs=8, space="PSUM"))
        sc = topstack.enter_context(tc.tile_pool(name="sc", bufs=32))

        # --- constants + state ------------------------------------------------
        g5_t = const.tile([128, 4, 5, N], f16)
        mask5_t = const.tile([128, 5, N], f32)
        x_t = const.tile([128, 2, 3, N], f32)
        rhs_t = const.tile([128, 2, 3, N], f32)
        r_t = const.tile([128, 2, 3, N], f32)
        p_t = const.tile([128, 2, 3, N], f32)
        acc_t = const.tile([128, 2, 3, N], f32)
        w1s_t = const.tile([18, 64], bf16)
        w2p_t = const.tile([128, 3, 64], bf16)
        w2q_t = const.tile([128, 3, 64], bf16)
        w3p_t = const.tile([128, 3, 2], bf16)
        w3q_t = const.tile([128, 3, 2], bf16)
        b1v_t = const.tile([64, 1], f32)
        b2v_t = const.tile([64, 1], f32)
        b3v_t = const.tile([2, 1], f32)

        nc.sync.dma_start(mask5_t[:], mask5_d[:, :, :])
        nc.sync.dma_start(x_t[:], xt_d[:, :, :, :])
        for k in range(4):
            nc.sync.dma_start(g5_t[:, k], g5_d[k])
        nc.sync.dma_start(w1s_t[:], w1s_d[:, :])
        nc.sync.dma_start(w2p_t[:], w2p_d[:, :, :])
        nc.sync.dma_start(w2q_t[:], w2q_d[:, :, :])
        nc.sync.dma_start(w3p_t[:], w3p_d[:, :, :])
        nc.sync.dma_start(w3q_t[:], w3q_d[:, :, :])
        nc.sync.dma_start(b1v_t[:], b1v_d[:, :])
        nc.sync.dma_start(b2v_t[:], b2v_d[:, :])
        nc.sync.dma_start(b3v_t[:], b3v_d[:, :])

        # DRAM staging for conv output (residual term), bf16
        dram = topstack.enter_context(tc.tile_pool(name="dram", bufs=1, space="DRAM"))
        o3stage = dram.tile([2, N, N], bf16)

        # --- denoiser conv (bf16, banded) ------------------------------------
        if DO_CONV:
            with tc.tile_pool(name="cstk", bufs=2) as cstk, \
                 tc.tile_pool(name="ch1", bufs=2) as ch1, \
                 tc.tile_pool(name="ch2", bufs=2) as ch2, \
                 tc.tile_pool(name="co3", bufs=2) as co3:
                for bd in range(NBAND):
                    s = bd * BAND
                    h1s, h1e = s - 2, s + BAND + 2        # h1 rows window [h1s,h1e) len 24
                    h2s, h2e = s - 1, s + BAND + 1        # h2 rows window len 22
                    v0 = max(0, -h1s)
                    v1 = 24 - max(0, h1e - N)
                    w0 = max(0, -h2s)
                    w1_ = 22 - max(0, h2e - N)

                    L1 = 24 * W2
                    L2L = 22 * W2
                    stk = cstk.tile([18, L1 + 8], bf16, tag="stk")
                    h1q = ch1.tile([128, L1 + 8], bf16, tag="h1q")
                    h2q = ch2.tile([128, L2L + 8], bf16, tag="h2q")
                    o3b = co3.tile([2, BAND * W2], bf16, tag="o3b")

                    span = (v1 - v0) * W2
                    nc.sync.dma_start(
                        stk[:18, 0:span],
                        stackh_d[:, (h1s + v0) * W2:(h1s + v0) * W2 + span],
                    )
                    # conv1
                    for j in range(0, span, 512):
                        n = min(512, span - j)
                        pt = ps.tile([128, 512], f32, tag="ps")
                        nc.tensor.matmul(pt[:64, :n], w1s_t[:, :], stk[:18, j:j + n],
                                         start=True, stop=True)
                        nc.scalar.activation(h1q[0:64, v0 * W2 + j:v0 * W2 + j + n],
                                             pt[:64, :n], AF.Relu, bias=b1v_t[:, :])
                    # zero invalid rows / pad cols / slack
                    if v0 > 0:
                        nc.gpsimd.memset(h1q[0:64, 0:v0 * W2], 0.0)
                    if v1 < 24:
                        nc.gpsimd.memset(h1q[0:64, v1 * W2:L1], 0.0)
                    nc.gpsimd.memset(h1q[0:64, L1:L1 + 8], 0.0)
                    h1v = h1q[0:64, 0:L1].rearrange("p (r x) -> p r x", x=W2)
                    nc.gpsimd.memset(h1v[:, :, 0:1], 0.0)
                    nc.gpsimd.memset(h1v[:, :, W2 - 1:W2], 0.0)
                    # dup shifted -1 into partitions 64:128 (dup[m]=base[m-1])
                    nc.sync.dma_start(h1q[64:128, 1:L1 + 8], h1q[0:64, 0:L1 + 7])
                    nc.gpsimd.memset(h1q[64:128, 0:1], 0.0)

                    # conv2: chunks over valid h2 rows [w0,w1)
                    for j in range(w0 * W2, w1_ * W2, 512):
                        n = min(512, w1_ * W2 - j)
                        pt = ps.tile([128, 512], f32, tag="ps")
                        k = 0
                        for dy in range(3):
                            nc.tensor.matmul(pt[:64, :n], w2p_t[:, dy, :],
                                             h1q[:, j + dy * W2:j + dy * W2 + n],
                                             start=(k == 0), stop=False)
                            k += 1
                        for dy in range(3):
                            k += 1
                            nc.tensor.matmul(pt[:64, :n], w2q_t[:, dy, :],
                                             h1q[:, j + dy * W2 + 1:j + dy * W2 + 1 + n],
                                             start=False, stop=(k == 6))
                        nc.scalar.activation(h2q[0:64, j:j + n], pt[:64, :n],
                                             AF.Relu, bias=b2v_t[:, :])
                    if w0 > 0:
                        nc.gpsimd.memset(h2q[0:64, 0:w0 * W2], 0.0)
                    if w1_ < 22:
                        nc.gpsimd.memset(h2q[0:64, w1_ * W2:L2L], 0.0)
                    nc.gpsimd.memset(h2q[0:64, L2L:L2L + 8], 0.0)
                    h2v = h2q[0:64, 0:L2L].rearrange("p (r x) -> p r x", x=W2)
                    nc.gpsimd.memset(h2v[:, :, 0:1], 0.0)
                    nc.gpsimd.memset(h2v[:, :, W2 - 1:W2], 0.0)
                    nc.sync.dma_start(h2q[64:128, 1:L2L + 8], h2q[0:64, 0:L2L + 7])
                    nc.gpsimd.memset(h2q[64:128, 0:1], 0.0)

                    # conv3: output rows [s, s+BAND)
                    for j in range(0, BAND * W2, 512):
                        n = min(512, BAND * W2 - j)
                        pt = ps.tile([128, 512], f32, tag="ps")
                        k = 0
                        for dy in range(3):
                            nc.tensor.matmul(pt[:2, :n], w3p_t[:, dy, :],
                                             h2q[:, j + dy * W2:j + dy * W2 + n],
                                             start=(k == 0), stop=False)
                            k += 1
                        for dy in range(3):
                            k += 1
                            nc.tensor.matmul(pt[:2, :n], w3q_t[:, dy, :],
                                             h2q[:, j + dy * W2 + 1:j + dy * W2 + 1 + n],
                                             start=False, stop=(k == 6))
                        nc.scalar.activation(o3b[0:2, j:j + n], pt[:2, :n],
                                             AF.Identity, bias=b3v_t[:, :])
                    o3v = o3b[0:2, :].rearrange("c (r x) -> c r x", x=W2)
                    nc.sync.dma_start(o3stage[:, s:s + BAND, :], o3v[:, :, 1:N + 1])

        # --- rhs partial: rhs = lam*(x + o3) ---------------------------------
        with tc.tile_pool(name="o3g", bufs=1) as o3g:
            o3t = o3g.tile([128, 2, 3, N], bf16)
            o3f = o3g.tile([128, 2, 3, N], f32)
            nc.gpsimd.memset(o3t[:, :, :, :], 0.0)
            if DO_CONV:
                for ch in range(2):
                    for t in range(2):
                        nc.sync.dma_start(
                            o3t[:, ch, t, :],
                            o3stage[ch, t * 128:(t + 1) * 128, :])
                    nc.sync.dma_start(
                        o3t[:64, ch, 2, :], o3stage[ch, 256:320, :])
            nc.vector.tensor_copy(o3f[:, :, :, :], o3t[:, :, :, :])
            for pl in range(2):
                nc.scalar.mul(rhs_t[:, pl], x_t[:, pl], L2LAM)
                nc.vector.scalar_tensor_tensor(
                    rhs_t[:, pl], o3f[:, pl], L2LAM, rhs_t[:, pl],
                    op0=AL.mult, op1=AL.add)

        # --- maps ------------------------------------------------------------
        mpool = topstack.enter_context(tc.tile_pool(name="maps", bufs=1))
        maps_t = mpool.tile([128, 12, 2, 3, N], f32)
        for c in range(12):
            nc.sync.dma_start(maps_t[:, c], mapst_d[:, c])

        # --- working pools ---------------------------------------------------
        work = topstack.enter_context(tc.tile_pool(name="work", bufs=4))
        tg_p = topstack.enter_context(tc.tile_pool(name="tg", bufs=2))
        td_p = topstack.enter_context(tc.tile_pool(name="td", bufs=6))
        scr_p = topstack.enter_context(tc.tile_pool(name="scr", bufs=1))

        def pass5(stat, rec, evac):
            """one complex 1D DFT pass over the packed-640 contraction.
            stat: [128,5,320] f16; evac(pl, mt, M, psum)"""
            for mt, (m0, M) in enumerate(TS):
                for pl in range(2):
                    pt = ps.tile([128, 512], f32, tag="ps")
                    g = g5_t[:, rec[pl]]
                    for j in range(5):
                        nc.tensor.matmul(
                            pt[:M, :N],
                            stat[:, j, m0:m0 + M],
                            g[:, j, :],
                            start=(j == 0), stop=(j == 4))
                    evac(pl, mt, M, pt)

        def plain_evac5(dst):
            def f(pl, mt, M, pt):
                ch, po = CHUNK[(pl, mt)]
                if po:
                    nc.vector.tensor_copy(dst[po:po + M, ch, :], pt[:M, :N])
                elif pl == 0:
                    nc.scalar.copy(dst[:M, ch, :], pt[:M, :N])
                else:
                    nc.vector.tensor_copy(dst[:M, ch, :], pt[:M, :N])
            return f

        def mask_evac5(dst):
            def f(pl, mt, M, pt):
                ch, po = CHUNK[(pl, mt)]
                # mask5 chunk 4 is plane-symmetric, so base-0 mask slice is
                # valid for the offset write too
                nc.vector.tensor_tensor(dst[po:po + M, ch, :], pt[:M, :N],
                                        mask5_t[:M, ch, :], AL.mult)
            return f

        def coil_mult(src, c):
            """A = maps[c] * src in 5-chunk f16 layout."""
            A = work.tile([128, 5, N], f16, tag="work")
            mr = maps_t[:, c, 0, 0:2, :]
            mi = maps_t[:, c, 1, 0:2, :]
            xr = src[:, 0, 0:2, :]
            xi = src[:, 1, 0:2, :]
            ta = tg_p.tile([128, 2, N], f32, tag="tg")
            tb = tg_p.tile([128, 2, N], f32, tag="tg")
            nc.gpsimd.tensor_tensor(ta[:], mr, xr, AL.mult)
            nc.gpsimd.tensor_tensor(tb[:], mi, xi, AL.mult)
            nc.gpsimd.tensor_tensor(A[:, 0:2, :], ta[:], tb[:], AL.subtract)
            tc_ = tg_p.tile([128, 2, N], f32, tag="tg")
            td = tg_p.tile([128, 2, N], f32, tag="tg")
            nc.gpsimd.tensor_tensor(tc_[:], mr, xi, AL.mult)
            nc.gpsimd.tensor_tensor(td[:], mi, xr, AL.mult)
            nc.gpsimd.tensor_tensor(A[:, 2:4, :], tc_[:], td[:], AL.add)
            # 64-row tails -> chunk 4 (imag half lands at partitions 64:128)
            mr2 = maps_t[0:64, c, 0, 2, :]
            mi2 = maps_t[0:64, c, 1, 2, :]
            xr2 = src[0:64, 0, 2, :]
            xi2 = src[0:64, 1, 2, :]
            e = tg_p.tile([64, 2, N], f32, tag="tg2")
            nc.gpsimd.tensor_tensor(e[:, 0, :], mr2, xr2, AL.mult)
            nc.gpsimd.tensor_tensor(e[:, 1, :], mi2, xi2, AL.mult)
            nc.gpsimd.tensor_tensor(A[0:64, 4, :], e[:, 0, :], e[:, 1, :],
                                    AL.subtract)
            f = tg_p.tile([64, 2, N], f32, tag="tg2")
            nc.gpsimd.tensor_tensor(f[:, 0, :], mr2, xi2, AL.mult)
            nc.gpsimd.tensor_tensor(f[:, 1, :], mi2, xr2, AL.mult)
            nc.vector.tensor_tensor(A[64:128, 4, :], f[:, 0, :], f[:, 1, :],
                                    AL.add)
            return A

        def final_pass5(stat, rec, c, acc):
            """last DFT pass: emit both plane groups per mt, then combine
            conj(maps[c]) * V into acc directly from PSUM."""
            for mt, (m0, M) in enumerate(TS):
                pts = []
                for pl in range(2):
                    pt = ps.tile([128, 512], f32, tag="ps")
                    g = g5_t[:, rec[pl]]
                    for j in range(5):
                        nc.tensor.matmul(
                            pt[:M, :N],
                            stat[:, j, m0:m0 + M],
                            g[:, j, :],
                            start=(j == 0), stop=(j == 4))
                    pts.append(pt)
                mr = maps_t[:M, c, 0, mt, :]
                mi = maps_t[:M, c, 1, mt, :]
                vr, vi = pts[0][:M, :N], pts[1][:M, :N]
                t1 = td_p.tile([128, N], f32, tag="td")
                t2 = td_p.tile([128, N], f32, tag="td")
                nc.vector.tensor_tensor(t1[:M], vr, mr, AL.mult)
                nc.vector.tensor_tensor(t2[:M], vi, mi, AL.mult)
                nc.vector.tensor_tensor(t1[:M], t1[:M], t2[:M], AL.add)
                nc.vector.tensor_tensor(acc[:M, 0, mt, :], acc[:M, 0, mt, :],
                                        t1[:M], AL.add)
                t3 = td_p.tile([128, N], f32, tag="td")
                t4 = td_p.tile([128, N], f32, tag="td")
                nc.vector.tensor_tensor(t3[:M], vi, mr, AL.mult)
                nc.vector.tensor_tensor(t4[:M], vr, mi, AL.mult)
                nc.vector.tensor_tensor(t3[:M], t3[:M], t4[:M], AL.subtract)
                nc.vector.tensor_tensor(acc[:M, 1, mt, :], acc[:M, 1, mt, :],
                                        t3[:M], AL.add)

        def emit_aop(src, acc):
            for pl in range(2):
                nc.scalar.mul(acc[:, pl], src[:, pl], L2LAM)
            for c in range(C):
                A = coil_mult(src, c)
                U1 = work.tile([128, 5, N], f16, tag="work")
                pass5(A, FWD, plain_evac5(U1))
                K2m = work.tile([128, 5, N], f16, tag="work")
                pass5(U1, FWD, mask_evac5(K2m))
                U2 = work.tile([128, 5, N], f16, tag="work")
                pass5(K2m, INV, plain_evac5(U2))
                final_pass5(U2, INV, c, acc)

        # --- adjoint into rhs -------------------------------------------------
        if DO_ADJ:
            with tc.tile_pool(name="kspp", bufs=1) as kspp:
                for c in range(C):
                    kt = kspp.tile([128, 2, 3, N], f32, tag="ksp")
                    nc.sync.dma_start(kt[:], kspt_d[:, c])
                    T = work.tile([128, 5, N], f16, tag="work")
                    nc.vector.tensor_tensor(T[:, 0:2, :], kt[:, 0, 0:2, :],
                                            mask5_t[:, 0:2, :], AL.mult)
                    nc.vector.tensor_tensor(T[:, 2:4, :], kt[:, 1, 0:2, :],
                                            mask5_t[:, 2:4, :], AL.mult)
                    nc.vector.tensor_tensor(T[0:64, 4, :], kt[0:64, 0, 2, :],
                                            mask5_t[0:64, 4, :], AL.mult)
                    nc.vector.tensor_tensor(T[64:128, 4, :], kt[0:64, 1, 2, :],
                                            mask5_t[0:64, 4, :], AL.mult)
                    U2 = work.tile([128, 5, N], f16, tag="work")
                    pass5(T, INV, plain_evac5(U2))
                    final_pass5(U2, INV, c, rhs_t)

        # --- CG ----------------------------------------------------------------
        AX = mybir.AxisListType
        onesf = const.tile([128, 128], f32)
        nc.gpsimd.memset(onesf[:], 1.0)
        ones_r = const.tile([128, 128], f32r)
        nc.vector.tensor_copy(ones_r[:], onesf[:])
        d8_p = topstack.enter_context(tc.tile_pool(name="d8", bufs=6))

        def emit_dot(a, b, out):
            """out[128,1] fp32 = sum(a*b) over both planes, broadcast to all
            partitions.  Partials -> [128,8] fp32r -> ones-matmul -> reduce."""
            p8a = d8_p.tile([128, 8], f32r, tag="d8")
            p8b = d8_p.tile([128, 8], f32r, tag="d8")
            for pl, p8 in ((0, p8a), (1, p8b)):
                scrap = scr_p.tile([128, 3, N], f32, tag="scrap")
                nc.vector.tensor_tensor(scrap[:], a[:, pl], b[:, pl], AL.mult)
                v8 = scrap[:].rearrange("p t n -> p (t n)").rearrange(
                    "p (a b) -> p a b", a=8)
                with nc.allow_low_precision(reason="fp32r dot partials"):
                    nc.vector.tensor_reduce(p8[:], v8, axis=AX.X, op=AL.add)
            with nc.allow_low_precision(reason="fp32r dot partials"):
                nc.vector.tensor_tensor(p8a[:], p8a[:], p8b[:], AL.add)
            s2 = ps.tile([128, 512], f32, tag="ps")
            nc.tensor.matmul(s2[:, 0:8], ones_r[:, :], p8a[:, :],
                             start=True, stop=True)
            nc.vector.tensor_reduce(out[:], s2[:, 0:8], axis=AX.X, op=AL.add)

        # r = rhs - Aop(x); p = r; rs = <r,r>
        emit_aop(x_t, acc_t)
        for pl in range(2):
            nc.vector.tensor_tensor(r_t[:, pl], rhs_t[:, pl], acc_t[:, pl],
                                    AL.subtract)
            nc.vector.tensor_copy(p_t[:, pl], r_t[:, pl])
        rs = sc.tile([128, 1], f32, tag="sc")
        emit_dot(r_t, r_t, rs)

        for it in range(CG_ITERS):
            emit_aop(p_t, acc_t)
            pap = sc.tile([128, 1], f32, tag="sc")
            emit_dot(p_t, acc_t, pap)
            rec = sc.tile([128, 1], f32, tag="sc")
            nc.vector.reciprocal(rec[:], pap[:])
            al = sc.tile([128, 1], f32, tag="sc")
            nc.vector.tensor_tensor(al[:], rs[:], rec[:], AL.mult)
            aln = sc.tile([128, 1], f32, tag="sc")
            nc.vector.tensor_scalar_mul(aln[:], al[:], -1.0)
            for pl in range(2):
                nc.vector.scalar_tensor_tensor(
                    x_t[:, pl], p_t[:, pl], al[:], x_t[:, pl],
                    op0=AL.mult, op1=AL.add)
                nc.vector.scalar_tensor_tensor(
                    r_t[:, pl], acc_t[:, pl], aln[:], r_t[:, pl],
                    op0=AL.mult, op1=AL.add)
            rsn = sc.tile([128, 1], f32, tag="sc")
            emit_dot(r_t, r_t, rsn)
            if it < CG_ITERS - 1:
                rrec = sc.tile([128, 1], f32, tag="sc")
                nc.vector.reciprocal(rrec[:], rs[:])
                be = sc.tile([128, 1], f32, tag="sc")
                nc.vector.tensor_tensor(be[:], rsn[:], rrec[:], AL.mult)
                for pl in range(2):
                    nc.vector.scalar_tensor_tensor(
                        p_t[:, pl], p_t[:, pl], be[:], r_t[:, pl],
                        op0=AL.mult, op1=AL.add)
            rs = rsn

        nc.sync.dma_start(xot_d[:, :, :, :], x_t[:])

    nc.compile()
    return nc


# ----------------------------------------------------------------------
# entry point
# ----------------------------------------------------------------------

def kernel(**inputs):
    from concourse.bass_utils import run_bass_kernel_spmd

    B = inputs["x"].shape[0]
    per_core = host_prep(inputs)

    if "nc" not in _cache:
        _cache["nc"] = build_program()
    nc = _cache["nc"]

    res = run_bass_kernel_spmd(nc, per_core, core_ids=list(range(B)))
    out = np.zeros((B, N, N, 2), np.float32)
    for b in range(B):
        xo = res.results[b]["xot"]          # [128,2,3,320]
        out[b, :, :, 0] = untile_rows(xo[:, 0])
        out[b, :, :, 1] = untile_rows(xo[:, 1])
    return out


# revision 14
# speedup vs baseline: 1.3740x; 1.0727x over previous
"""MoDL recon (one unroll) Trainium2 Bass kernel.

B=8 batch elements sharded 1-per-core across 8 NeuronCores (pure data
parallel).  Per core: SENSE adjoint, 3-layer CNN denoiser, and 6 CG
iterations on the SENSE normal operator, for C=12 coils on 320x320 complex
images.

All 2D centered FFTs are computed as dense DFT matmuls on the tensor engine
using the two-pass "image stationary" trick:
    U  = MM(X, G)  = X.T @ G        (G symmetric centered DFT matrix)
    K2 = MM(U, G)  = U.T @ G = G X G = fftc(X)
which needs no explicit transposes.  ifftc uses conj(G).

DFT matmuls run in fp16 (1 cycle/row on the PE vs 2+ for fp32r) with the
complex contraction packed into 640 rows = 5 chunks of K=128:
chunks [re rows 0:128 | re 128:256 | im 0:128 | im 128:256 | re-tail::im-tail]
against four precomputed stacked G matrices (fwd/inv x out-real/out-imag).
5 accumulating matmuls per PSUM tile instead of 6, no K=64 stragglers.

Host-side work is layout only: de-interleaving re/im planes, row-tiling
[320,320] -> [128,3,320], packing the stacked DFT matrices / mask, packing
conv weights, and building the conv1 im2col stack.
"""

import os
import numpy as np
import ml_dtypes

N = 320
NT = 3
TS = [(0, 128), (128, 128), (256, 64)]   # (row0, rows) per row-tile
C = int(os.environ.get("K_COILS", "12"))
CG_ITERS = int(os.environ.get("K_CG", "6"))
DO_CONV = os.environ.get("K_CONV", "1") == "1"
DO_ADJ = os.environ.get("K_ADJ", "1") == "1"
L2LAM = 0.05

# conv band structure
BAND = 20
NBAND = N // BAND
W2 = N + 2  # padded width 322

# (plane, mt) -> (chunk, partition offset) in the 5-chunk stationary layout
CHUNK = {(0, 0): (0, 0), (0, 1): (1, 0), (0, 2): (4, 0),
         (1, 0): (2, 0), (1, 1): (3, 0), (1, 2): (4, 64)}

_cache = {}


# ----------------------------------------------------------------------
# host-side helpers
# ----------------------------------------------------------------------

def centered_dft_matrix(n):
    F = np.fft.fft(np.eye(n), norm="ortho", axis=0)
    s = np.fft.fftshift(np.eye(n), axes=0)
    si = np.fft.ifftshift(np.eye(n), axes=0)
    return (s @ F @ si).astype(np.complex128)


def tile_rows(x):
    """[..., 320, n] -> [128, ..., 3, n] with rows r = t*128+p, zero pad."""
    lead = x.shape[:-2]
    n = x.shape[-1]
    xp = np.zeros(lead + (384, n), dtype=x.dtype)
    xp[..., :320, :] = x
    xp = xp.reshape(lead + (3, 128, n))          # [..., t, p, n]
    # move p to front
    xp = np.moveaxis(xp, -2, 0)                  # [128, ..., t, n]
    return np.ascontiguousarray(xp)


def untile_rows(x):
    """[128, 3, n] -> [320, n]"""
    # x[p, t, n] -> rows t*128+p
    out = np.transpose(x, (1, 0, 2)).reshape(384, x.shape[-1])
    return out[:320]


def stack5(p0, p1, dtype):
    """[320,320] plane-0 + plane-1 matrices -> [128, 5, 320] chunk layout."""
    out = np.zeros((128, 5, N), np.float64)
    out[:, 0, :] = p0[0:128]
    out[:, 1, :] = p0[128:256]
    out[:, 2, :] = p1[0:128]
    out[:, 3, :] = p1[128:256]
    out[0:64, 4, :] = p0[256:320]
    out[64:128, 4, :] = p1[256:320]
    return np.ascontiguousarray(out.astype(dtype))


def host_prep(inputs):
    x = inputs["x"]
    maps = inputs["maps"]
    masks = inputs["masks"]
    ksp = inputs["ksp"]
    w1, b1 = inputs["w1"], inputs["b1"]
    w2, b2 = inputs["w2"], inputs["b2"]
    w3, b3 = inputs["w3"], inputs["b3"]
    B = x.shape[0]

    G = centered_dft_matrix(N)
    f16 = np.float16
    # stacked DFT matrices: moving operands for the packed-640 contraction
    g5 = np.stack([
        stack5(G.real, -G.imag, f16),   # fwd, real out
        stack5(G.imag, G.real, f16),    # fwd, imag out
        stack5(G.real, G.imag, f16),    # inv, real out
        stack5(-G.imag, G.real, f16),   # inv, imag out
    ])                                   # [4, 128, 5, 320]

    # conv weights
    w1s = np.zeros((18, 64), np.float32)
    for dy in range(3):
        for dx in range(3):
            off = dy * 3 + dx
            for ci in range(2):
                w1s[off * 2 + ci, :] = w1[:, ci, dy, dx]
    w2p = np.zeros((128, 3, 64), np.float32)
    w2q = np.zeros((128, 3, 64), np.float32)
    w3p = np.zeros((128, 3, 2), np.float32)
    w3q = np.zeros((128, 3, 2), np.float32)
    for dy in range(3):
        w2p[0:64, dy, :] = w2[:, :, dy, 1].T     # center tap reads base half
        w2p[64:128, dy, :] = w2[:, :, dy, 0].T   # left tap reads dup (-1) half
        w2q[0:64, dy, :] = w2[:, :, dy, 2].T     # right tap: base half at +1
        w3p[0:64, dy, :] = w3[:, :, dy, 1].T
        w3p[64:128, dy, :] = w3[:, :, dy, 0].T
        w3q[0:64, dy, :] = w3[:, :, dy, 2].T

    bf = ml_dtypes.bfloat16
    shared = {
        "g5": g5,
        "w1s": w1s.astype(bf),
        "w2p": np.ascontiguousarray(w2p.astype(bf)),
        "w2q": np.ascontiguousarray(w2q.astype(bf)),
        "w3p": np.ascontiguousarray(w3p.astype(bf)),
        "w3q": np.ascontiguousarray(w3q.astype(bf)),
        "b1v": b1.reshape(64, 1).astype(np.float32),
        "b2v": b2.reshape(64, 1).astype(np.float32),
        "b3v": b3.reshape(2, 1).astype(np.float32),
    }

    per_core = []
    for b in range(B):
        xpl = np.transpose(x[b], (2, 0, 1)).astype(np.float32)      # [2,320,320]
        mpl = np.transpose(maps[b], (3, 0, 1, 2)).astype(np.float16)  # [2,12,320,320]
        mpl = np.transpose(mpl, (1, 0, 2, 3))                        # [12,2,320,320]
        kpl = np.transpose(ksp[b], (3, 0, 1, 2)).astype(np.float16)
        kpl = np.transpose(kpl, (1, 0, 2, 3))

        # mask in 5-chunk layout (chunks 2,3,4-hi duplicate for the imag plane)
        m = masks[b].astype(np.float32)
        mask5 = np.zeros((128, 5, N), np.float32)
        mask5[:, 0, :] = m[0:128]
        mask5[:, 1, :] = m[128:256]
        mask5[:, 2, :] = m[0:128]
        mask5[:, 3, :] = m[128:256]
        mask5[0:64, 4, :] = m[256:320]
        mask5[64:128, 4, :] = m[256:320]

        # conv1 stack: [18, 320*322] bf16
        xq = np.zeros((2, W2, W2 + 2), np.float32)
        xq[:, 1:321, 2:322] = xpl
        stack = np.zeros((18, N * W2), np.float32)
        for dy in range(3):
            for dx in range(3):
                off = dy * 3 + dx
                for ci in range(2):
                    stack[off * 2 + ci] = xq[ci, dy:dy + N, dx:dx + W2].reshape(-1)

        per_core.append({
            "xt": tile_rows(xpl),                         # [128,2,3,320]
            "mapst": tile_rows(mpl),                      # [128,12,2,3,320]
            "kspt": tile_rows(kpl),
            "mask5": mask5,
            "stackh": stack.astype(bf),
            **shared,
        })
    return per_core


# ----------------------------------------------------------------------
# device program
# ----------------------------------------------------------------------

def build_program():
    import concourse.bass as bass
    import concourse.mybir as mybir
    import concourse.tile as tile
    from concourse import bacc, bass_isa
    from contextlib import ExitStack

    f32 = mybir.dt.float32
    f32r = mybir.dt.float32r
    f16 = mybir.dt.float16
    bf16 = mybir.dt.bfloat16
    AL = mybir.AluOpType
    AF = mybir.ActivationFunctionType

    nc = bacc.Bacc("TRN2", target_bir_lowering=False)

    # DRAM tensors
    xt_d = nc.dram_tensor("xt", [128, 2, 3, N], f32, kind="ExternalInput")
    mapst_d = nc.dram_tensor("mapst", [128, 12, 2, 3, N], f16, kind="ExternalInput")
    kspt_d = nc.dram_tensor("kspt", [128, 12, 2, 3, N], f16, kind="ExternalInput")
    mask5_d = nc.dram_tensor("mask5", [128, 5, N], f32, kind="ExternalInput")
    g5_d = nc.dram_tensor("g5", [4, 128, 5, N], f16, kind="ExternalInput")
    stackh_d = nc.dram_tensor("stackh", [18, N * W2], bf16, kind="ExternalInput")
    w1s_d = nc.dram_tensor("w1s", [18, 64], bf16, kind="ExternalInput")
    w2p_d = nc.dram_tensor("w2p", [128, 3, 64], bf16, kind="ExternalInput")
    w2q_d = nc.dram_tensor("w2q", [128, 3, 64], bf16, kind="ExternalInput")
    w3p_d = nc.dram_tensor("w3p", [128, 3, 2], bf16, kind="ExternalInput")
    w3q_d = nc.dram_tensor("w3q", [128, 3, 2], bf16, kind="ExternalInput")
    b1v_d = nc.dram_tensor("b1v", [64, 1], f32, kind="ExternalInput")
    b2v_d = nc.dram_tensor("b2v", [64, 1], f32, kind="ExternalInput")
    b3v_d = nc.dram_tensor("b3v", [2, 1], f32, kind="ExternalInput")
    xot_d = nc.dram_tensor("xot", [128, 2, 3, N], f32, kind="ExternalOutput")

    FWD = (0, 1)   # g5 indices for the two out planes
    INV = (2, 3)

    with tile.TileContext(nc) as tc, ExitStack() as topstack:
        const = topstack.enter_context(tc.tile_pool(name="const", bufs=1))
        ps = topstack.enter_context(tc.tile_pool(name="ps", bufs=8, space="PSUM"))
        sc = topstack.enter_context(tc.tile_pool(name="sc", bufs=32))

        # --- constants + state ------------------------------------------------
        g5_t = const.tile([128, 4, 5, N], f16)
        mask5_t = const.tile([128, 5, N], f32)
        x_t = const.tile([128, 2, 3, N], f32)
        rhs_t = const.tile([128, 2, 3, N], f32)
        r_t = const.tile([128, 2, 3, N], f32)
        acc_t = const.tile([128, 2, 3, N], f32)
        w1s_t = const.tile([18, 64], bf16)
        w2p_t = const.tile([128, 3, 64], bf16)
        w2q_t = const.tile([128, 3, 64], bf16)
        w3p_t = const.tile([128, 3, 2], bf16)
        w3q_t = const.tile([128, 3, 2], bf16)
        b1v_t = const.tile([64, 1], f32)
        b2v_t = const.tile([64, 1], f32)
        b3v_t = const.tile([2, 1], f32)

        nc.sync.dma_start(mask5_t[:], mask5_d[:, :, :])
        nc.sync.dma_start(x_t[:], xt_d[:, :, :, :])
        for k in range(4):
            nc.sync.dma_start(g5_t[:, k], g5_d[k])
        nc.sync.dma_start(w1s_t[:], w1s_d[:, :])
        nc.sync.dma_start(w2p_t[:], w2p_d[:, :, :])
        nc.sync.dma_start(w2q_t[:], w2q_d[:, :, :])
        nc.sync.dma_start(w3p_t[:], w3p_d[:, :, :])
        nc.sync.dma_start(w3q_t[:], w3q_d[:, :, :])
        nc.sync.dma_start(b1v_t[:], b1v_d[:, :])
        nc.sync.dma_start(b2v_t[:], b2v_d[:, :])
        nc.sync.dma_start(b3v_t[:], b3v_d[:, :])

        # DRAM staging for conv output (residual term), bf16
        dram = topstack.enter_context(tc.tile_pool(name="dram", bufs=1, space="DRAM"))
        o3stage = dram.tile([2, N, N], bf16)

        # --- denoiser conv (bf16, banded) ------------------------------------
        if DO_CONV:
            with tc.tile_pool(name="cstk", bufs=2) as cstk, \
                 tc.tile_pool(name="ch1", bufs=2) as ch1, \
                 tc.tile_pool(name="ch2", bufs=2) as ch2, \
                 tc.tile_pool(name="co3", bufs=2) as co3:
                for bd in range(NBAND):
                    s = bd * BAND
                    h1s, h1e = s - 2, s + BAND + 2        # h1 rows window [h1s,h1e) len 24
                    h2s, h2e = s - 1, s + BAND + 1        # h2 rows window len 22
                    v0 = max(0, -h1s)
                    v1 = 24 - max(0, h1e - N)
                    w0 = max(0, -h2s)
                    w1_ = 22 - max(0, h2e - N)

                    L1 = 24 * W2
                    L2L = 22 * W2
                    stk = cstk.tile([18, L1 + 8], bf16, tag="stk")
                    h1q = ch1.tile([128, L1 + 8], bf16, tag="h1q")
                    h2q = ch2.tile([128, L2L + 8], bf16, tag="h2q")
                    o3b = co3.tile([2, BAND * W2], bf16, tag="o3b")

                    span = (v1 - v0) * W2
                    nc.sync.dma_start(
                        stk[:18, 0:span],
                        stackh_d[:, (h1s + v0) * W2:(h1s + v0) * W2 + span],
                    )
                    # conv1
                    for j in range(0, span, 512):
                        n = min(512, span - j)
                        pt = ps.tile([128, 512], f32, tag="ps")
                        nc.tensor.matmul(pt[:64, :n], w1s_t[:, :], stk[:18, j:j + n],
                                         start=True, stop=True)
                        nc.scalar.activation(h1q[0:64, v0 * W2 + j:v0 * W2 + j + n],
                                             pt[:64, :n], AF.Relu, bias=b1v_t[:, :])
                    # zero invalid rows / pad cols / slack
                    if v0 > 0:
                        nc.gpsimd.memset(h1q[0:64, 0:v0 * W2], 0.0)
                    if v1 < 24:
                        nc.gpsimd.memset(h1q[0:64, v1 * W2:L1], 0.0)
                    nc.gpsimd.memset(h1q[0:64, L1:L1 + 8], 0.0)
                    h1v = h1q[0:64, 0:L1].rearrange("p (r x) -> p r x", x=W2)
                    nc.gpsimd.memset(h1v[:, :, 0:1], 0.0)
                    nc.gpsimd.memset(h1v[:, :, W2 - 1:W2], 0.0)
                    # dup shifted -1 into partitions 64:128 (dup[m]=base[m-1])
                    nc.sync.dma_start(h1q[64:128, 1:L1 + 8], h1q[0:64, 0:L1 + 7])
                    nc.gpsimd.memset(h1q[64:128, 0:1], 0.0)

                    # conv2: chunks over valid h2 rows [w0,w1)
                    for j in range(w0 * W2, w1_ * W2, 512):
                        n = min(512, w1_ * W2 - j)
                        pt = ps.tile([128, 512], f32, tag="ps")
                        k = 0
                        for dy in range(3):
                            nc.tensor.matmul(pt[:64, :n], w2p_t[:, dy, :],
                                             h1q[:, j + dy * W2:j + dy * W2 + n],
                                             start=(k == 0), stop=False)
                            k += 1
                        for dy in range(3):
                            k += 1
                            nc.tensor.matmul(pt[:64, :n], w2q_t[:, dy, :],
                                             h1q[:, j + dy * W2 + 1:j + dy * W2 + 1 + n],
                                             start=False, stop=(k == 6))
                        nc.scalar.activation(h2q[0:64, j:j + n], pt[:64, :n],
                                             AF.Relu, bias=b2v_t[:, :])
                    if w0 > 0:
                        nc.gpsimd.memset(h2q[0:64, 0:w0 * W2], 0.0)
                    if w1_ < 22:
                        nc.gpsimd.memset(h2q[0:64, w1_ * W2:L2L], 0.0)
                    nc.gpsimd.memset(h2q[0:64, L2L:L2L + 8], 0.0)
                    h2v = h2q[0:64, 0:L2L].rearrange("p (r x) -> p r x", x=W2)
                    nc.gpsimd.memset(h2v[:, :, 0:1], 0.0)
                    nc.gpsimd.memset(h2v[:, :, W2 - 1:W2], 0.0)
                    nc.sync.dma_start(h2q[64:128, 1:L2L + 8], h2q[0:64, 0:L2L + 7])
                    nc.gpsimd.memset(h2q[64:128, 0:1], 0.0)

                    # conv3: output rows [s, s+BAND)
                    for j in range(0, BAND * W2, 512):
                        n = min(512, BAND * W2 - j)
                        pt = ps.tile([128, 512], f32, tag="ps")
                        k = 0
                        for dy in range(3):
                            nc.tensor.matmul(pt[:2, :n], w3p_t[:, dy, :],
                                             h2q[:, j + dy * W2:j + dy * W2 + n],
                                             start=(k == 0), stop=False)
                            k += 1
                        for dy in range(3):
                            k += 1
                            nc.tensor.matmul(pt[:2, :n], w3q_t[:, dy, :],
                                             h2q[:, j + dy * W2 + 1:j + dy * W2 + 1 + n],
                                             start=False, stop=(k == 6))
                        nc.scalar.activation(o3b[0:2, j:j + n], pt[:2, :n],
                                             AF.Identity, bias=b3v_t[:, :])
                    o3v = o3b[0:2, :].rearrange("c (r x) -> c r x", x=W2)
                    nc.sync.dma_start(o3stage[:, s:s + BAND, :], o3v[:, :, 1:N + 1])

        # --- rhs partial: rhs = lam*(x + o3) ---------------------------------
        with tc.tile_pool(name="o3g", bufs=1) as o3g:
            o3t = o3g.tile([128, 2, 3, N], bf16)
            o3f = o3g.tile([128, 2, 3, N], f32)
            nc.gpsimd.memset(o3t[:, :, :, :], 0.0)
            if DO_CONV:
                for ch in range(2):
                    for t in range(2):
                        nc.sync.dma_start(
                            o3t[:, ch, t, :],
                            o3stage[ch, t * 128:(t + 1) * 128, :])
                    nc.sync.dma_start(
                        o3t[:64, ch, 2, :], o3stage[ch, 256:320, :])
            nc.vector.tensor_copy(o3f[:, :, :, :], o3t[:, :, :, :])
            for pl in range(2):
                nc.scalar.mul(rhs_t[:, pl], x_t[:, pl], L2LAM)
                nc.vector.scalar_tensor_tensor(
                    rhs_t[:, pl], o3f[:, pl], L2LAM, rhs_t[:, pl],
                    op0=AL.mult, op1=AL.add)

        # --- maps ------------------------------------------------------------
        mpool = topstack.enter_context(tc.tile_pool(name="maps", bufs=1))
        maps_t = mpool.tile([128, 12, 2, 3, N], f16)
        for c in range(12):
            nc.sync.dma_start(maps_t[:, c], mapst_d[:, c])

        # --- working pools ---------------------------------------------------
        work = topstack.enter_context(tc.tile_pool(name="work", bufs=8))
        vv_p = topstack.enter_context(tc.tile_pool(name="vv", bufs=2))
        tg_p = topstack.enter_context(tc.tile_pool(name="tg", bufs=2))
        td_p = topstack.enter_context(tc.tile_pool(name="td", bufs=4))
        scr_p = topstack.enter_context(tc.tile_pool(name="scr", bufs=2))

        def pass5(stat, rec, evac):
            """one complex 1D DFT pass over the packed-640 contraction.
            stat: [128,5,320] f16; evac(pl, mt, M, psum)"""
            for mt, (m0, M) in enumerate(TS):
                for pl in range(2):
                    pt = ps.tile([128, 512], f32, tag="ps")
                    g = g5_t[:, rec[pl]]
                    for j in range(5):
                        nc.tensor.matmul(
                            pt[:M, :N],
                            stat[:, j, m0:m0 + M],
                            g[:, j, :],
                            start=(j == 0), stop=(j == 4))
                    evac(pl, mt, M, pt)

        def plain_evac5(dst):
            def f(pl, mt, M, pt):
                ch, po = CHUNK[(pl, mt)]
                if po:
                    # partition-offset write: only DVE is verified for this
                    nc.vector.tensor_copy(dst[po:po + M, ch, :], pt[:M, :N])
                else:
                    nc.scalar.copy(dst[:M, ch, :], pt[:M, :N])
            return f

        def mask_evac5(dst):
            def f(pl, mt, M, pt):
                ch, po = CHUNK[(pl, mt)]
                # mask5 chunk 4 is plane-symmetric, so base-0 mask slice is
                # valid for the offset write too
                nc.vector.tensor_tensor(dst[po:po + M, ch, :], pt[:M, :N],
                                        mask5_t[:M, ch, :], AL.mult)
            return f

        def coil_mult(src, c):
            """A = maps[c] * src in 5-chunk f16 layout."""
            A = work.tile([128, 5, N], f16, tag="work")
            mr = maps_t[:, c, 0, 0:2, :]
            mi = maps_t[:, c, 1, 0:2, :]
            xr = src[:, 0, 0:2, :]
            xi = src[:, 1, 0:2, :]
            ta = tg_p.tile([128, 2, N], f32, tag="tg")
            tb = tg_p.tile([128, 2, N], f32, tag="tg")
            nc.gpsimd.tensor_tensor(ta[:], mr, xr, AL.mult)
            nc.gpsimd.tensor_tensor(tb[:], mi, xi, AL.mult)
            nc.gpsimd.tensor_tensor(A[:, 0:2, :], ta[:], tb[:], AL.subtract)
            tc_ = tg_p.tile([128, 2, N], f32, tag="tg")
            td = tg_p.tile([128, 2, N], f32, tag="tg")
            nc.gpsimd.tensor_tensor(tc_[:], mr, xi, AL.mult)
            nc.gpsimd.tensor_tensor(td[:], mi, xr, AL.mult)
            nc.gpsimd.tensor_tensor(A[:, 2:4, :], tc_[:], td[:], AL.add)
            # 64-row tails -> chunk 4 (imag half lands at partitions 64:128)
            mr2 = maps_t[0:64, c, 0, 2, :]
            mi2 = maps_t[0:64, c, 1, 2, :]
            xr2 = src[0:64, 0, 2, :]
            xi2 = src[0:64, 1, 2, :]
            e = tg_p.tile([64, 2, N], f32, tag="tg2")
            nc.gpsimd.tensor_tensor(e[:, 0, :], mr2, xr2, AL.mult)
            nc.gpsimd.tensor_tensor(e[:, 1, :], mi2, xi2, AL.mult)
            nc.gpsimd.tensor_tensor(A[0:64, 4, :], e[:, 0, :], e[:, 1, :],
                                    AL.subtract)
            f = tg_p.tile([64, 2, N], f32, tag="tg2")
            nc.gpsimd.tensor_tensor(f[:, 0, :], mr2, xi2, AL.mult)
            nc.gpsimd.tensor_tensor(f[:, 1, :], mi2, xr2, AL.mult)
            nc.vector.tensor_tensor(A[64:128, 4, :], f[:, 0, :], f[:, 1, :],
                                    AL.add)
            return A

        def final_pass5(stat, rec, c, acc):
            """last DFT pass: evacuate V to SBUF fast (frees PSUM banks),
            then combine conj(maps[c]) * V into acc with whole-plane slabs
            split across the vector (real) and gpsimd (imag) engines."""
            V = vv_p.tile([128, 2, 3, N], f16, tag="vv")
            # t2 pad rows are never written by the evacs; stale f16 bits there
            # could be NaN and 0*NaN would poison the padded maps product
            nc.gpsimd.memset(V[64:128, :, 2, :], 0.0)
            for mt, (m0, M) in enumerate(TS):
                for pl in range(2):
                    pt = ps.tile([128, 512], f32, tag="ps")
                    g = g5_t[:, rec[pl]]
                    for j in range(5):
                        nc.tensor.matmul(
                            pt[:M, :N],
                            stat[:, j, m0:m0 + M],
                            g[:, j, :],
                            start=(j == 0), stop=(j == 4))
                    nc.scalar.copy(V[:M, pl, mt, :], pt[:M, :N])
            mr = maps_t[:, c, 0]
            mi = maps_t[:, c, 1]
            vr = V[:, 0]
            vi = V[:, 1]
            t1 = td_p.tile([128, 3, N], f16, tag="td")
            t2 = td_p.tile([128, 3, N], f16, tag="td")
            nc.vector.tensor_tensor(t1[:], vr, mr, AL.mult)
            nc.vector.tensor_tensor(t2[:], vi, mi, AL.mult)
            nc.vector.tensor_tensor(t1[:], t1[:], t2[:], AL.add)
            nc.vector.tensor_tensor(acc[:, 0], acc[:, 0], t1[:], AL.add)
            t3 = td_p.tile([128, 3, N], f16, tag="td2")
            t4 = td_p.tile([128, 3, N], f16, tag="td2")
            nc.gpsimd.tensor_tensor(t3[:], vi, mr, AL.mult)
            nc.gpsimd.tensor_tensor(t4[:], vr, mi, AL.mult)
            nc.gpsimd.tensor_tensor(t3[:], t3[:], t4[:], AL.subtract)
            nc.gpsimd.tensor_tensor(acc[:, 1], acc[:, 1], t3[:], AL.add)

        def emit_aop(src, acc):
            for pl in range(2):
                nc.scalar.mul(acc[:, pl], src[:, pl], L2LAM)
            for c in range(C):
                A = coil_mult(src, c)
                U1 = work.tile([128, 5, N], f16, tag="work")
                pass5(A, FWD, plain_evac5(U1))
                K2m = work.tile([128, 5, N], f16, tag="work")
                pass5(U1, FWD, mask_evac5(K2m))
                U2 = work.tile([128, 5, N], f16, tag="work")
                pass5(K2m, INV, plain_evac5(U2))
                final_pass5(U2, INV, c, acc)

        # --- adjoint into rhs -------------------------------------------------
        if DO_ADJ:
            with tc.tile_pool(name="kspp", bufs=1) as kspp:
                for c in range(C):
                    kt = kspp.tile([128, 2, 3, N], f16, tag="ksp")
                    nc.sync.dma_start(kt[:], kspt_d[:, c])
                    T = work.tile([128, 5, N], f16, tag="work")
                    nc.vector.tensor_tensor(T[:, 0:2, :], kt[:, 0, 0:2, :],
                                            mask5_t[:, 0:2, :], AL.mult)
                    nc.vector.tensor_tensor(T[:, 2:4, :], kt[:, 1, 0:2, :],
                                            mask5_t[:, 2:4, :], AL.mult)
                    nc.vector.tensor_tensor(T[0:64, 4, :], kt[0:64, 0, 2, :],
                                            mask5_t[0:64, 4, :], AL.mult)
                    nc.vector.tensor_tensor(T[64:128, 4, :], kt[0:64, 1, 2, :],
                                            mask5_t[0:64, 4, :], AL.mult)
                    U2 = work.tile([128, 5, N], f16, tag="work")
                    pass5(T, INV, plain_evac5(U2))
                    final_pass5(U2, INV, c, rhs_t)

        # --- CG ----------------------------------------------------------------
        onesf = const.tile([128, 128], f32)
        nc.gpsimd.memset(onesf[:], 1.0)
        d8_p = topstack.enter_context(tc.tile_pool(name="d8", bufs=6))
        pp_p = topstack.enter_context(tc.tile_pool(name="pp", bufs=2))

        def emit_dot(a, b, out, eng):
            """out[128,1] fp32 = sum(a*b) over both planes broadcast to all
            partitions, via fused multiply+row-reduce then a ones-matmul."""
            scrap = scr_p.tile([128, 2, 3, N], f32, tag="scrap")
            acc1 = d8_p.tile([128, 1], f32, tag="d8")
            eng.scalar_tensor_tensor(scrap[:], a[:], 1.0, b[:],
                                     op0=AL.mult, op1=AL.mult,
                                     accum_out=acc1[:])
            s2 = ps.tile([128, 512], f32, tag="ps")
            nc.tensor.matmul(s2[:, 0:1], onesf[:, :], acc1[:, :],
                             start=True, stop=True)
            nc.vector.tensor_copy(out[:], s2[:, 0:1])

        # r = rhs - Aop(x); p = r; rs = <r,r>
        emit_aop(x_t, acc_t)
        p_cur = pp_p.tile([128, 2, 3, N], f32, tag="pp")
        nc.vector.scalar_tensor_tensor(r_t[:], rhs_t[:], 1.0, acc_t[:],
                                       op0=AL.mult, op1=AL.subtract)
        nc.vector.tensor_copy(p_cur[:], r_t[:])
        rs = sc.tile([128, 1], f32, tag="sc")
        emit_dot(r_t, r_t, rs, nc.vector)

        for it in range(CG_ITERS):
            last = it == CG_ITERS - 1
            emit_aop(p_cur, acc_t)
            # <p, Ap> and <Ap, Ap> have no mutual dependency: run on separate
            # engines right after acc completes
            pap = sc.tile([128, 1], f32, tag="sc")
            emit_dot(p_cur, acc_t, pap, nc.vector)
            rec = sc.tile([128, 1], f32, tag="sc")
            nc.vector.reciprocal(rec[:], pap[:])
            al = sc.tile([128, 1], f32, tag="sc")
            nc.vector.tensor_tensor(al[:], rs[:], rec[:], AL.mult)
            if not last:
                apap = sc.tile([128, 1], f32, tag="sc")
                emit_dot(acc_t, acc_t, apap, nc.vector)
                aln = sc.tile([128, 1], f32, tag="sc")
                nc.vector.tensor_scalar_mul(aln[:], al[:], -1.0)
                # rsn = al^2 * <Ap,Ap> - rs  (exact-CG identity; avoids a
                # third full-size dot on the critical path)
                t5 = sc.tile([128, 1], f32, tag="sc")
                nc.vector.tensor_tensor(t5[:], al[:], al[:], AL.mult)
                t6 = sc.tile([128, 1], f32, tag="sc")
                nc.vector.tensor_tensor(t6[:], t5[:], apap[:], AL.mult)
                rsn = sc.tile([128, 1], f32, tag="sc")
                nc.vector.tensor_tensor(rsn[:], t6[:], rs[:], AL.subtract)
                rrec = sc.tile([128, 1], f32, tag="sc")
                nc.vector.reciprocal(rrec[:], rs[:])
                be = sc.tile([128, 1], f32, tag="sc")
                nc.vector.tensor_tensor(be[:], rsn[:], rrec[:], AL.mult)
                # r_new = r - al*Ap ; p_new = be*p + r_new (split engines)
                nc.vector.scalar_tensor_tensor(
                    r_t[:, 0], acc_t[:, 0], aln[:], r_t[:, 0],
                    op0=AL.mult, op1=AL.add)
                nc.vector.scalar_tensor_tensor(
                    r_t[:, 1], acc_t[:, 1], aln[:], r_t[:, 1],
                    op0=AL.mult, op1=AL.add)
                p_new = pp_p.tile([128, 2, 3, N], f32, tag="pp")
                nc.vector.scalar_tensor_tensor(
                    p_new[:, 0], p_cur[:, 0], be[:], r_t[:, 0],
                    op0=AL.mult, op1=AL.add)
                nc.vector.scalar_tensor_tensor(
                    p_new[:, 1], p_cur[:, 1], be[:], r_t[:, 1],
                    op0=AL.mult, op1=AL.add)
            # x += al*p  (reads the old p; off the boundary critical path)
            nc.vector.scalar_tensor_tensor(
                x_t[:, 0], p_cur[:, 0], al[:], x_t[:, 0],
                op0=AL.mult, op1=AL.add)
            nc.vector.scalar_tensor_tensor(
                x_t[:, 1], p_cur[:, 1], al[:], x_t[:, 1],
                op0=AL.mult, op1=AL.add)
            if not last:
                p_cur = p_new
                rs = rsn

        nc.sync.dma_start(xot_d[:, :, :, :], x_t[:])

    nc.compile()
    return nc


# ----------------------------------------------------------------------
# entry point
# ----------------------------------------------------------------------

def kernel(**inputs):
    from concourse.bass_utils import run_bass_kernel_spmd

    B = inputs["x"].shape[0]
    per_core = host_prep(inputs)

    if "nc" not in _cache:
        _cache["nc"] = build_program()
    nc = _cache["nc"]

    res = run_bass_kernel_spmd(nc, per_core, core_ids=list(range(B)))
    out = np.zeros((B, N, N, 2), np.float32)
    for b in range(B):
        xo = res.results[b]["xot"]          # [128,2,3,320]
        out[b, :, :, 0] = untile_rows(xo[:, 0])
        out[b, :, :, 1] = untile_rows(xo[:, 1])
    return out
